# revision 18
# baseline (speedup 1.0000x reference)
"""CIF (Continuous Integrate-and-Fire) model kernel for Trainium2, 8 NeuronCores.

Strategy (data-parallel over batch, 2 examples per core):

  Phase 1 (device): pre-sigmoid alpha logits.
      hidden [2,T,H] -> PE-transpose to [h,t] tiles -> depthwise conv as 3
      diagonal matmuls (residual folded into center tap) -> fused
      bias+relu (ACT/DVE, PSUM->SBUF) -> H-reduction matmul with lin_w as
      the stationary operand -> z [2,T].

  Host: alphas = sigmoid(z + lin_b); exact sequential f32 integrate-and-fire
      scan (bit-identical semantics to the reference scan given these
      alphas); derive fire times, per-time slot indices and gather weights.
      Only O(B*T) scalar work.

  Phase 2 (device): cif_output rows. For each 128-slot output tile, gather
      its 384-row time window of hidden via indirect DMA, multiply with the
      host-built weight matrix W [384,128] (3 accumulating f32 matmuls x 2
      column halves) -> PSUM [128,1024] -> DMA to output.

  fires / cif_length come from the host scan (they are tiny and must be
  bit-consistent with the fire decisions used for the gather).
"""

import numpy as np

B, T, H = 16, 2048, 1024
NCORES = 8
BPC = B // NCORES            # batches per core
NHT = H // 128               # h tiles
TCHUNK = 512
NTC = T // TCHUNK
WIN = 384                    # phase-2 time window rows per slot tile
NCH = WIN // 128             # chunks per window
SLOT_TILE = 128
LPAD = 1152                  # padded output slots (9 tiles); actual L ~ 1056
NST = LPAD // SLOT_TILE

_CACHE = {}


def _build_phase1():
    import concourse.bass as bass
    from concourse import bacc, mybir
    from concourse.tile import TileContext
    from concourse.masks import make_identity

    f32 = mybir.dt.float32
    nc = bacc.Bacc()
    hid = nc.declare_dram_parameter("hidden", [BPC, T, H], f32, isOutput=False)
    taps = nc.declare_dram_parameter("taps", [NHT, 3, 128, 128], f32, isOutput=False)
    cbias = nc.declare_dram_parameter("cbias", [NHT, 128], f32, isOutput=False)
    linw = nc.declare_dram_parameter("linw", [NHT, 128], f32, isOutput=False)
    zout = nc.declare_dram_parameter("z", [BPC, T], f32, isOutput=True)

    with TileContext(nc) as tc:
        with (
            tc.tile_pool(name="consts", bufs=1) as consts,
            tc.tile_pool(name="xt", bufs=2) as xtp,
            tc.tile_pool(name="work", bufs=3) as work,
            tc.tile_pool(name="zp", bufs=2) as zp,
            tc.tile_pool(name="pst", bufs=2, space="PSUM") as pstp,
            tc.tile_pool(name="psc", bufs=2, space="PSUM") as pscp,
            tc.tile_pool(name="psz", bufs=2, space="PSUM") as pszp,
        ):
            ident = consts.tile([128, 128], f32)
            make_identity(nc, ident)
            taps_sb = consts.tile([128, NHT, 3, 128], f32)
            nc.sync.dma_start(out=taps_sb, in_=taps.rearrange("n k p j -> p n k j"))
            cb_sb = consts.tile([128, NHT], f32)
            nc.sync.dma_start(out=cb_sb, in_=cbias.rearrange("n p -> p n"))
            lw_sb = consts.tile([128, NHT], f32)
            nc.sync.dma_start(out=lw_sb, in_=linw.rearrange("n p -> p n"))

            # ACT (153.6 Gelem/s) vs DVE (122.9) both run PSUM->SBUF at 1x;
            # split the two full-array passes 5:9 / 4:9 so they finish together
            cp_idx = [0]

            def pick_engine():
                i = cp_idx[0]
                cp_idx[0] += 1
                return nc.scalar if (i % 9) < 5 else nc.vector

            for b in range(BPC):
                xts = []
                for ht in range(NHT):
                    nblk = T // 128
                    xnat = work.tile([128, nblk, 128], f32, tag="xnat")
                    for blk in range(nblk):
                        nc.sync.dma_start(
                            out=xnat[:, blk, :],
                            in_=hid[b, blk * 128:(blk + 1) * 128,
                                    ht * 128:(ht + 1) * 128],
                        )
                    xt = xtp.tile([128, T + 2], f32, tag=f"xt{ht}")
                    nc.vector.memset(xt[:, 0:1], 0.0)
                    nc.vector.memset(xt[:, T + 1:T + 2], 0.0)
                    for blk in range(nblk):
                        pst = pstp.tile([128, 128], f32, tag="pst")
                        # transpose as a plain matmul (out = in.T @ I);
                        # is_transpose mode faults on this runtime
                        nc.tensor.matmul(
                            out=pst, lhsT=xnat[:, blk, :], rhs=ident,
                            start=True, stop=True)
                        dst = xt[:, 1 + blk * 128:1 + (blk + 1) * 128]
                        eng = pick_engine()
                        if eng is nc.vector:
                            nc.vector.tensor_copy(dst, pst)
                        else:
                            nc.scalar.copy(dst, pst)
                    xts.append(xt)

                zsb = zp.tile([1, T], f32, tag="zsb")
                for tci in range(NTC):
                    psz = pszp.tile([1, TCHUNK], f32, tag="psz")
                    for ht in range(NHT):
                        psc = pscp.tile([128, TCHUNK], f32, tag="psc")
                        for k in range(3):
                            nc.tensor.matmul(
                                out=psc,
                                lhsT=taps_sb[:, ht, k, :],
                                rhs=xts[ht][:, tci * TCHUNK + k:
                                            tci * TCHUNK + k + TCHUNK],
                                start=(k == 0),
                                stop=(k == 2),
                            )
                        relu = work.tile([128, TCHUNK], f32, tag="relu")
                        if pick_engine() is nc.scalar:
                            nc.scalar.activation(
                                out=relu, in_=psc,
                                func=mybir.ActivationFunctionType.Relu,
                                bias=cb_sb[:, ht:ht + 1], scale=1.0)
                        else:
                            nc.vector.tensor_scalar(
                                out=relu, in0=psc,
                                scalar1=cb_sb[:, ht:ht + 1], scalar2=0.0,
                                op0=mybir.AluOpType.add,
                                op1=mybir.AluOpType.max)
                        nc.tensor.matmul(
                            out=psz,
                            lhsT=lw_sb[:, ht:ht + 1],
                            rhs=relu,
                            start=(ht == 0),
                            stop=(ht == NHT - 1),
                        )
                    nc.vector.tensor_copy(
                        zsb[:, tci * TCHUNK:(tci + 1) * TCHUNK], psz)
                nc.sync.dma_start(out=zout[b:b + 1, :], in_=zsb)
    nc.compile()
    return nc


def _build_phase2():
    import concourse.bass as bass
    from concourse import bacc, mybir
    from concourse.tile import TileContext

    f32 = mybir.dt.float32
    i32 = mybir.dt.int32
    nc = bacc.Bacc()
    hid = nc.declare_dram_parameter("hidden", [BPC * T, H], f32, isOutput=False)
    widx = nc.declare_dram_parameter("widx", [BPC, NST, NCH, 128], i32,
                                     isOutput=False)
    wmat = nc.declare_dram_parameter("wmat", [BPC, NST, WIN, 128], f32,
                                     isOutput=False)
    out = nc.declare_dram_parameter("out", [BPC, LPAD, H], f32, isOutput=True)

    with TileContext(nc) as tc:
        with (
            tc.tile_pool(name="idx", bufs=3) as idxp,
            tc.tile_pool(name="wt", bufs=3) as wtp,
            tc.tile_pool(name="hrows", bufs=4) as hp,
            tc.tile_pool(name="osb", bufs=2) as osbp,
            tc.tile_pool(name="pso", bufs=2, space="PSUM") as psp,
        ):
            for b in range(BPC):
                for st in range(NST):
                    idxt = idxp.tile([128, NCH], i32, tag="idx")
                    nc.sync.dma_start(
                        out=idxt, in_=widx[b, st].rearrange("c p -> p c"))
                    wt = wtp.tile([128, NCH, 128], f32, tag="wt")
                    nc.sync.dma_start(
                        out=wt,
                        in_=wmat[b, st].rearrange("(c p) j -> p c j", p=128))
                    pso = psp.tile([128, H], f32, tag="pso")
                    for c in range(NCH):
                        hrows = hp.tile([128, H], f32, tag="hrows")
                        nc.gpsimd.indirect_dma_start(
                            out=hrows,
                            out_offset=None,
                            in_=hid[:, :],
                            in_offset=bass.IndirectOffsetOnAxis(
                                ap=idxt[:, c:c + 1], axis=0),
                        )
                        for half in range(2):
                            nc.tensor.matmul(
                                out=pso[:, half * 512:(half + 1) * 512],
                                lhsT=wt[:, c, :],
                                rhs=hrows[:, half * 512:(half + 1) * 512],
                                start=(c == 0),
                                stop=(c == NCH - 1),
                            )
                    out_sb = osbp.tile([128, H], f32, tag="osb")
                    if st % 2 == 0:
                        nc.vector.tensor_copy(out_sb, pso)
                    else:
                        nc.scalar.copy(out_sb, pso)
                    nc.sync.dma_start(
                        out=out[b, st * 128:(st + 1) * 128, :], in_=out_sb)
    nc.compile()
    return nc


def _phase1_inputs(conv_w, conv_b, lin_w):
    taps = np.zeros((NHT, 3, 128, 128), np.float32)
    for ht in range(NHT):
        for k in range(3):
            d = conv_w[ht * 128:(ht + 1) * 128, 0, k].astype(np.float32).copy()
            if k == 1:
                d = d + np.float32(1.0)  # residual folded into center tap
            np.fill_diagonal(taps[ht, k], d)
    cbias = conv_b.astype(np.float32).reshape(NHT, 128)
    linw = lin_w.astype(np.float32).reshape(NHT, 128)
    return taps, cbias, linw


def _exact_scan(alphas):
    """Reference-semantics sequential f32 integrate-and-fire scan.

    Returns fires [B,T] f32 (pre-reset integrate values), fire mask m [B,T],
    i_prev [B,T] (carry before each step).
    """
    b, t = alphas.shape
    integ = np.zeros(b, np.float32)
    fires = np.zeros((b, t), np.float32)
    m = np.zeros((b, t), bool)
    i_prev = np.zeros((b, t), np.float32)
    one = np.float32(1.0)
    for s in range(t):
        i_prev[:, s] = integ
        integ = (integ + alphas[:, s]).astype(np.float32)
        fires[:, s] = integ
        f = integ >= one
        m[:, s] = f
        integ = np.where(f, (integ - one).astype(np.float32), integ)
    return fires, m, i_prev


def _phase2_inputs(alphas, m, i_prev):
    """Build per-(batch, slot-tile) gather indices and weight matrices."""
    one = np.float32(1.0)
    widx = np.zeros((B, NST, NCH, 128), np.int32)
    wmat = np.zeros((B, NST, WIN, 128), np.float32)
    dist = (one - i_prev).astype(np.float32)
    rem = (alphas - dist).astype(np.float32)
    for b in range(B):
        base = (b % BPC) * T  # row base inside this core's local hidden slice
        ft = np.flatnonzero(m[b])
        nf = len(ft)
        # slot index receiving the "cur" weight at each time: fires before t
        n1 = np.concatenate([[0], np.cumsum(m[b].astype(np.int64))[:-1]])
        w_cur = np.where(m[b], dist[b], alphas[b]).astype(np.float32)
        ntile_b = (nf + SLOT_TILE - 1) // SLOT_TILE
        for st in range(NST):
            if st >= ntile_b:
                widx[b, st] = base  # harmless row, weights stay zero
                continue
            lo = int(ft[SLOT_TILE * st - 1]) if st > 0 else 0
            hi = int(ft[min(SLOT_TILE * (st + 1), nf) - 1])
            if hi - lo + 1 > WIN:
                raise RuntimeError(
                    f"slot tile span {hi - lo + 1} exceeds window {WIN}")
            rows = lo + np.arange(WIN)
            valid = rows < T
            widx[b, st] = (base + np.minimum(rows, T - 1)).reshape(NCH, 128)
            tv = rows[valid]
            # cur contribution -> slot n1[t]
            j1 = n1[tv] - SLOT_TILE * st
            sel = (j1 >= 0) & (j1 < SLOT_TILE) & (n1[tv] < nf)
            wmat[b, st, np.flatnonzero(valid)[sel], j1[sel]] += w_cur[tv[sel]]
            # rem contribution at fire times -> slot n1[t] + 1
            fsel = m[b][tv]
            j2 = n1[tv] + 1 - SLOT_TILE * st
            sel2 = fsel & (j2 >= 0) & (j2 < SLOT_TILE) & (n1[tv] + 1 < nf)
            wmat[b, st, np.flatnonzero(valid)[sel2], j2[sel2]] += rem[b][tv[sel2]]
    return widx, wmat


def _get_programs():
    if "p1" not in _CACHE:
        _CACHE["p1"] = _build_phase1()
        _CACHE["p2"] = _build_phase2()
    return _CACHE["p1"], _CACHE["p2"]


_LAST = {}


def kernel(hidden, conv_w, conv_b, lin_w, lin_b):
    import os
    from concourse.bass_utils import run_bass_kernel_spmd

    trace = bool(int(os.environ.get("KERNEL_TRACE", "0")))
    hidden = np.ascontiguousarray(np.asarray(hidden, np.float32))
    nc1, nc2 = _get_programs()
    core_ids = list(range(NCORES))

    taps, cbias, linw = _phase1_inputs(
        np.asarray(conv_w), np.asarray(conv_b), np.asarray(lin_w))
    in_maps1 = [
        {"hidden": hidden[c * BPC:(c + 1) * BPC], "taps": taps,
         "cbias": cbias, "linw": linw}
        for c in range(NCORES)
    ]
    r1 = run_bass_kernel_spmd(nc1, in_maps1, core_ids, trace=trace)
    _LAST["p1"] = r1
    res1 = r1.results
    z = np.concatenate([res1[c]["z"] for c in range(NCORES)], 0)  # [B,T]

    zb = (z + np.asarray(lin_b, np.float32)[0]).astype(np.float32)
    alphas = (1.0 / (1.0 + np.exp(-zb.astype(np.float64)))).astype(np.float32)

    fires, m, i_prev = _exact_scan(alphas)
    cif_length = alphas.sum(axis=1, dtype=np.float32)
    max_label_len = int(np.round(cif_length).max())

    widx, wmat = _phase2_inputs(alphas, m, i_prev)
    in_maps2 = [
        {"hidden": hidden[c * BPC:(c + 1) * BPC].reshape(BPC * T, H),
         "widx": widx[c * BPC:(c + 1) * BPC],
         "wmat": wmat[c * BPC:(c + 1) * BPC]}
        for c in range(NCORES)
    ]
    r2 = run_bass_kernel_spmd(nc2, in_maps2, core_ids, trace=trace)
    _LAST["p2"] = r2
    res2 = r2.results

    L = max_label_len
    cif_output = np.zeros((B, L, H), np.float32)
    for c in range(NCORES):
        dev = res2[c]["out"]  # [BPC, LPAD, H]
        for bl in range(BPC):
            cif_output[c * BPC + bl] = dev[bl, :L, :]
    return cif_output, cif_length, fires


# revision 20
# speedup vs baseline: 1.0326x; 1.0326x over previous
"""CIF (Continuous Integrate-and-Fire) model kernel for Trainium2, 8 NeuronCores.

Strategy (data-parallel over batch, 2 examples per core):

  Phase 1 (device): pre-sigmoid alpha logits.
      hidden [2,T,H] -> PE-transpose to [h,t] tiles -> depthwise conv as 3
      diagonal matmuls (residual folded into center tap) -> fused
      bias+relu (ACT/DVE, PSUM->SBUF) -> H-reduction matmul with lin_w as
      the stationary operand -> z [2,T].

  Host: alphas = sigmoid(z + lin_b); exact sequential f32 integrate-and-fire
      scan (bit-identical semantics to the reference scan given these
      alphas); derive fire times, per-time slot indices and gather weights.
      Only O(B*T) scalar work.

  Phase 2 (device): cif_output rows. For each 128-slot output tile, gather
      its 384-row time window of hidden via indirect DMA, multiply with the
      host-built weight matrix W [384,128] (3 accumulating f32 matmuls x 2
      column halves) -> PSUM [128,1024] -> DMA to output.

  fires / cif_length come from the host scan (they are tiny and must be
  bit-consistent with the fire decisions used for the gather).
"""

import numpy as np

B, T, H = 16, 2048, 1024
NCORES = 8
BPC = B // NCORES            # batches per core
NHT = H // 128               # h tiles
TCHUNK = 512
NTC = T // TCHUNK
WIN = 384                    # phase-2 time window rows per slot tile
NCH = WIN // 128             # chunks per window
SLOT_TILE = 128
LPAD = 1152                  # padded output slots (9 tiles); actual L ~ 1056
NST = LPAD // SLOT_TILE

_CACHE = {}


def _build_phase1():
    import concourse.bass as bass
    from concourse import bacc, mybir
    from concourse.tile import TileContext
    from concourse.masks import make_identity

    f32 = mybir.dt.float32
    nc = bacc.Bacc()
    hid = nc.declare_dram_parameter("hidden", [BPC, T, H], f32, isOutput=False)
    taps = nc.declare_dram_parameter("taps", [NHT, 3, 128, 128], f32, isOutput=False)
    cbias = nc.declare_dram_parameter("cbias", [NHT, 128], f32, isOutput=False)
    linw = nc.declare_dram_parameter("linw", [NHT, 128], f32, isOutput=False)
    zout = nc.declare_dram_parameter("z", [BPC, T], f32, isOutput=True)

    with TileContext(nc) as tc:
        with (
            tc.tile_pool(name="consts", bufs=1) as consts,
            tc.tile_pool(name="xt", bufs=2) as xtp,
            tc.tile_pool(name="work", bufs=4) as work,
            tc.tile_pool(name="zp", bufs=2) as zp,
            tc.tile_pool(name="pst", bufs=3, space="PSUM") as pstp,
            tc.tile_pool(name="psc", bufs=3, space="PSUM") as pscp,
            tc.tile_pool(name="psz", bufs=2, space="PSUM") as pszp,
        ):
            ident = consts.tile([128, 128], f32)
            make_identity(nc, ident)
            taps_sb = consts.tile([128, NHT, 3, 128], f32)
            nc.sync.dma_start(out=taps_sb, in_=taps.rearrange("n k p j -> p n k j"))
            cb_sb = consts.tile([128, NHT], f32)
            nc.sync.dma_start(out=cb_sb, in_=cbias.rearrange("n p -> p n"))
            lw_sb = consts.tile([128, NHT], f32)
            nc.sync.dma_start(out=lw_sb, in_=linw.rearrange("n p -> p n"))

            # ACT (153.6 Gelem/s) vs DVE (122.9) both run PSUM->SBUF at 1x;
            # split the two full-array passes 5:9 / 4:9 so they finish together
            cp_idx = [0]

            def pick_engine():
                i = cp_idx[0]
                cp_idx[0] += 1
                return nc.scalar if (i % 9) < 5 else nc.vector

            for b in range(BPC):
                xts = []
                for ht in range(NHT):
                    nblk = T // 128
                    xnat = work.tile([128, nblk, 128], f32, tag="xnat")
                    for blk in range(nblk):
                        nc.sync.dma_start(
                            out=xnat[:, blk, :],
                            in_=hid[b, blk * 128:(blk + 1) * 128,
                                    ht * 128:(ht + 1) * 128],
                        )
                    xt = xtp.tile([128, T + 2], f32, tag=f"xt{ht}")
                    nc.vector.memset(xt[:, 0:1], 0.0)
                    nc.vector.memset(xt[:, T + 1:T + 2], 0.0)
                    for blk in range(nblk):
                        pst = pstp.tile([128, 128], f32, tag="pst")
                        # transpose as a plain matmul (out = in.T @ I);
                        # is_transpose mode faults on this runtime
                        nc.tensor.matmul(
                            out=pst, lhsT=xnat[:, blk, :], rhs=ident,
                            start=True, stop=True)
                        dst = xt[:, 1 + blk * 128:1 + (blk + 1) * 128]
                        eng = pick_engine()
                        if eng is nc.vector:
                            nc.vector.tensor_copy(dst, pst)
                        else:
                            nc.scalar.copy(dst, pst)
                    xts.append(xt)

                zsb = zp.tile([1, T], f32, tag="zsb")
                for tci in range(NTC):
                    psz = pszp.tile([1, TCHUNK], f32, tag="psz")
                    # software-pipeline: the psz reduce-matmul for h-tile ht
                    # issues after conv(ht+1), giving the relu (ACT/DVE) a
                    # full conv's latency to land before the in-order PE
                    # needs it
                    pending = None
                    for ht in range(NHT):
                        psc = pscp.tile([128, TCHUNK], f32, tag="psc")
                        for k in range(3):
                            nc.tensor.matmul(
                                out=psc,
                                lhsT=taps_sb[:, ht, k, :],
                                rhs=xts[ht][:, tci * TCHUNK + k:
                                            tci * TCHUNK + k + TCHUNK],
                                start=(k == 0),
                                stop=(k == 2),
                            )
                        relu = work.tile([128, TCHUNK], f32, tag="relu")
                        if pick_engine() is nc.scalar:
                            nc.scalar.activation(
                                out=relu, in_=psc,
                                func=mybir.ActivationFunctionType.Relu,
                                bias=cb_sb[:, ht:ht + 1], scale=1.0)
                        else:
                            nc.vector.tensor_scalar(
                                out=relu, in0=psc,
                                scalar1=cb_sb[:, ht:ht + 1], scalar2=0.0,
                                op0=mybir.AluOpType.add,
                                op1=mybir.AluOpType.max)
                        if pending is not None:
                            p_relu, p_ht = pending
                            nc.tensor.matmul(
                                out=psz, lhsT=lw_sb[:, p_ht:p_ht + 1],
                                rhs=p_relu, start=(p_ht == 0),
                                stop=False)
                        pending = (relu, ht)
                    p_relu, p_ht = pending
                    nc.tensor.matmul(
                        out=psz, lhsT=lw_sb[:, p_ht:p_ht + 1],
                        rhs=p_relu, start=False, stop=True)
                    nc.vector.tensor_copy(
                        zsb[:, tci * TCHUNK:(tci + 1) * TCHUNK], psz)
                nc.sync.dma_start(out=zout[b:b + 1, :], in_=zsb)
    nc.compile()
    return nc


def _build_phase2():
    import concourse.bass as bass
    from concourse import bacc, mybir
    from concourse.tile import TileContext

    f32 = mybir.dt.float32
    i32 = mybir.dt.int32
    nc = bacc.Bacc()
    hid = nc.declare_dram_parameter("hidden", [BPC * T, H], f32, isOutput=False)
    widx = nc.declare_dram_parameter("widx", [BPC, NST, NCH, 128], i32,
                                     isOutput=False)
    wmat = nc.declare_dram_parameter("wmat", [BPC, NST, WIN, 128], f32,
                                     isOutput=False)
    out = nc.declare_dram_parameter("out", [BPC, LPAD, H], f32, isOutput=True)

    with TileContext(nc) as tc:
        with (
            tc.tile_pool(name="idx", bufs=3) as idxp,
            tc.tile_pool(name="wt", bufs=3) as wtp,
            tc.tile_pool(name="hrows", bufs=4) as hp,
            tc.tile_pool(name="osb", bufs=2) as osbp,
            tc.tile_pool(name="pso", bufs=2, space="PSUM") as psp,
        ):
            for b in range(BPC):
                for st in range(NST):
                    idxt = idxp.tile([128, NCH], i32, tag="idx")
                    nc.sync.dma_start(
                        out=idxt, in_=widx[b, st].rearrange("c p -> p c"))
                    wt = wtp.tile([128, NCH, 128], f32, tag="wt")
                    nc.sync.dma_start(
                        out=wt,
                        in_=wmat[b, st].rearrange("(c p) j -> p c j", p=128))
                    pso = psp.tile([128, H], f32, tag="pso")
                    for c in range(NCH):
                        hrows = hp.tile([128, H], f32, tag="hrows")
                        nc.gpsimd.indirect_dma_start(
                            out=hrows,
                            out_offset=None,
                            in_=hid[:, :],
                            in_offset=bass.IndirectOffsetOnAxis(
                                ap=idxt[:, c:c + 1], axis=0),
                        )
                        for half in range(2):
                            nc.tensor.matmul(
                                out=pso[:, half * 512:(half + 1) * 512],
                                lhsT=wt[:, c, :],
                                rhs=hrows[:, half * 512:(half + 1) * 512],
                                start=(c == 0),
                                stop=(c == NCH - 1),
                            )
                    out_sb = osbp.tile([128, H], f32, tag="osb")
                    if st % 2 == 0:
                        nc.vector.tensor_copy(out_sb, pso)
                    else:
                        nc.scalar.copy(out_sb, pso)
                    nc.sync.dma_start(
                        out=out[b, st * 128:(st + 1) * 128, :], in_=out_sb)
    nc.compile()
    return nc


def _phase1_inputs(conv_w, conv_b, lin_w):
    taps = np.zeros((NHT, 3, 128, 128), np.float32)
    for ht in range(NHT):
        for k in range(3):
            d = conv_w[ht * 128:(ht + 1) * 128, 0, k].astype(np.float32).copy()
            if k == 1:
                d = d + np.float32(1.0)  # residual folded into center tap
            np.fill_diagonal(taps[ht, k], d)
    cbias = conv_b.astype(np.float32).reshape(NHT, 128)
    linw = lin_w.astype(np.float32).reshape(NHT, 128)
    return taps, cbias, linw


def _exact_scan(alphas):
    """Reference-semantics sequential f32 integrate-and-fire scan.

    Returns fires [B,T] f32 (pre-reset integrate values), fire mask m [B,T],
    i_prev [B,T] (carry before each step).
    """
    b, t = alphas.shape
    integ = np.zeros(b, np.float32)
    fires = np.zeros((b, t), np.float32)
    m = np.zeros((b, t), bool)
    i_prev = np.zeros((b, t), np.float32)
    one = np.float32(1.0)
    for s in range(t):
        i_prev[:, s] = integ
        integ = (integ + alphas[:, s]).astype(np.float32)
        fires[:, s] = integ
        f = integ >= one
        m[:, s] = f
        integ = np.where(f, (integ - one).astype(np.float32), integ)
    return fires, m, i_prev


def _phase2_inputs(alphas, m, i_prev):
    """Build per-(batch, slot-tile) gather indices and weight matrices."""
    one = np.float32(1.0)
    widx = np.zeros((B, NST, NCH, 128), np.int32)
    wmat = np.zeros((B, NST, WIN, 128), np.float32)
    dist = (one - i_prev).astype(np.float32)
    rem = (alphas - dist).astype(np.float32)
    for b in range(B):
        base = (b % BPC) * T  # row base inside this core's local hidden slice
        ft = np.flatnonzero(m[b])
        nf = len(ft)
        # slot index receiving the "cur" weight at each time: fires before t
        n1 = np.concatenate([[0], np.cumsum(m[b].astype(np.int64))[:-1]])
        w_cur = np.where(m[b], dist[b], alphas[b]).astype(np.float32)
        ntile_b = (nf + SLOT_TILE - 1) // SLOT_TILE
        for st in range(NST):
            if st >= ntile_b:
                widx[b, st] = base  # harmless row, weights stay zero
                continue
            lo = int(ft[SLOT_TILE * st - 1]) if st > 0 else 0
            hi = int(ft[min(SLOT_TILE * (st + 1), nf) - 1])
            if hi - lo + 1 > WIN:
                raise RuntimeError(
                    f"slot tile span {hi - lo + 1} exceeds window {WIN}")
            rows = lo + np.arange(WIN)
            valid = rows < T
            widx[b, st] = (base + np.minimum(rows, T - 1)).reshape(NCH, 128)
            tv = rows[valid]
            # cur contribution -> slot n1[t]
            j1 = n1[tv] - SLOT_TILE * st
            sel = (j1 >= 0) & (j1 < SLOT_TILE) & (n1[tv] < nf)
            wmat[b, st, np.flatnonzero(valid)[sel], j1[sel]] += w_cur[tv[sel]]
            # rem contribution at fire times -> slot n1[t] + 1
            fsel = m[b][tv]
            j2 = n1[tv] + 1 - SLOT_TILE * st
            sel2 = fsel & (j2 >= 0) & (j2 < SLOT_TILE) & (n1[tv] + 1 < nf)
            wmat[b, st, np.flatnonzero(valid)[sel2], j2[sel2]] += rem[b][tv[sel2]]
    return widx, wmat


def _get_programs():
    if "p1" not in _CACHE:
        _CACHE["p1"] = _build_phase1()
        _CACHE["p2"] = _build_phase2()
    return _CACHE["p1"], _CACHE["p2"]


_LAST = {}


def kernel(hidden, conv_w, conv_b, lin_w, lin_b):
    import os
    from concourse.bass_utils import run_bass_kernel_spmd

    trace = bool(int(os.environ.get("KERNEL_TRACE", "0")))
    hidden = np.ascontiguousarray(np.asarray(hidden, np.float32))
    nc1, nc2 = _get_programs()
    core_ids = list(range(NCORES))

    taps, cbias, linw = _phase1_inputs(
        np.asarray(conv_w), np.asarray(conv_b), np.asarray(lin_w))
    in_maps1 = [
        {"hidden": hidden[c * BPC:(c + 1) * BPC], "taps": taps,
         "cbias": cbias, "linw": linw}
        for c in range(NCORES)
    ]
    r1 = run_bass_kernel_spmd(nc1, in_maps1, core_ids, trace=trace)
    _LAST["p1"] = r1
    res1 = r1.results
    z = np.concatenate([res1[c]["z"] for c in range(NCORES)], 0)  # [B,T]

    zb = (z + np.asarray(lin_b, np.float32)[0]).astype(np.float32)
    alphas = (1.0 / (1.0 + np.exp(-zb.astype(np.float64)))).astype(np.float32)

    fires, m, i_prev = _exact_scan(alphas)
    cif_length = alphas.sum(axis=1, dtype=np.float32)
    max_label_len = int(np.round(cif_length).max())

    widx, wmat = _phase2_inputs(alphas, m, i_prev)
    in_maps2 = [
        {"hidden": hidden[c * BPC:(c + 1) * BPC].reshape(BPC * T, H),
         "widx": widx[c * BPC:(c + 1) * BPC],
         "wmat": wmat[c * BPC:(c + 1) * BPC]}
        for c in range(NCORES)
    ]
    r2 = run_bass_kernel_spmd(nc2, in_maps2, core_ids, trace=trace)
    _LAST["p2"] = r2
    res2 = r2.results

    L = max_label_len
    cif_output = np.zeros((B, L, H), np.float32)
    for c in range(NCORES):
        dev = res2[c]["out"]  # [BPC, LPAD, H]
        for bl in range(BPC):
            cif_output[c * BPC + bl] = dev[bl, :L, :]
    return cif_output, cif_length, fires
